# revision 1
# baseline (speedup 1.0000x reference)
"""Trainium2 Bass kernel for sparse-attention AttnBlock.

Math: the reference gathers, per query q, K=64 key/value vectors at
attendable_indices[q, :], masks invalid slots to -inf, softmaxes over the
64 slots and mixes the gathered values.  Because softmax slots with equal
indices contribute identical exp-terms, the whole sparse attention is
exactly equivalent to a *dense* attention over all 4096 keys with an
integer multiplicity matrix:

    M[q, j]   = #{k : attendable_indices[q,k] == j and valid[q,k] == 1}
    W[j, q]   = M[q, j] * exp(K_j . Q_q)            (no max-subtraction
    Z[q]      = sum_j W[j, q]                        needed: logits are O(1))
    attn[q]   = (sum_j W[j, q] * V_j) / Z[q]

M is shared across batch and heads and is built on the host from the int32
index inputs (pure index preprocessing).  On device everything is dense
matmul + exp — no gather.

Sharding: 8 cores = batch (2) x query-quarter (4).  Each core computes
GroupNorm for its batch image, full K/V projections (all 4 heads), Q for
its 1024 queries, attention, and the output projection + residual for its
exact output slab [256, 1024] — concatenated on the host, no reduction.
"""

import numpy as np
import ml_dtypes
from contextlib import ExitStack

import concourse.bass as bass
import concourse.bacc as bacc
import concourse.mybir as mybir
import concourse.tile as tile
from concourse.bass_utils import run_bass_kernel_spmd

B, C, HI, WI = 2, 256, 64, 64
NQ = HI * WI          # 4096 spatial positions
HEADS, D = 4, 64
GROUPS, EPS = 32, 1e-6
NCORES = 8
QS = NQ // (NCORES // B)   # 1024 queries per core
NJC = NQ // 128            # 32 key chunks of 128
NQP = QS // 512            # 2 query halves of 512
CPG = C // GROUPS          # 8 channels per group
GPC = 128 // CPG           # 16 groups per 128-channel chunk

f32, bf16 = mybir.dt.float32, mybir.dt.bfloat16
FT = mybir.ActivationFunctionType
OP = mybir.AluOpType

_CACHE = {}
USE_TTR = False
USE_SQRT = True
USE_RECIP = True


def _split_dma_waits(nc):
    """The axon/bass2jax walrus invocation codegens Tile sync directly and
    rejects DMA instructions carrying more than one sync-wait.  Move extra
    waits onto a same-engine NoOp inserted right before the DMA (the SEQ
    executes it first, preserving ordering)."""
    f = nc.m.functions[0]
    for bb in f.blocks:
        i = 0
        insts = bb.instructions
        while i < len(insts):
            ins = insts[i]
            si = ins.sync_info
            if (str(ins.opcode) in ("DMACopy", "DMATranspose")
                    and si is not None and len(si.on_wait) > 1):
                nop = mybir.InstNoOp(
                    name=nc.get_next_instruction_name(), ins=[], outs=[])
                nop.engine = ins.engine
                nop.sync_info = mybir.SyncInfo(
                    on_wait=list(si.on_wait)[:-1], on_update=[])
                si.on_wait = [si.on_wait[-1]]
                nc.register_instruction(nop)
                insts.insert(i, nop)
                i += 1
            i += 1


def _build_program(stage="full", reps=1):
    nc = bacc.Bacc(None, target_bir_lowering=False, debug=False)
    _declare_and_emit(nc, stage, reps)
    nc.compile()
    _split_dma_waits(nc)
    return nc


def _declare_and_emit(nc, stage, reps=1):

    # ---- DRAM I/O (per-core) ----
    x_d = nc.dram_tensor("x", [2, 128, NQ], f32, kind="ExternalInput")
    xq_d = nc.dram_tensor("xq", [2, 128, QS], f32, kind="ExternalInput")
    mt_d = nc.dram_tensor("mt", [NQ, QS], bf16, kind="ExternalInput")
    wqT_d = nc.dram_tensor("wqT", [2, 128, C], bf16, kind="ExternalInput")
    wkT_d = nc.dram_tensor("wkT", [2, 128, C], bf16, kind="ExternalInput")
    wvT_d = nc.dram_tensor("wvT", [2, 128, C], bf16, kind="ExternalInput")
    woT_d = nc.dram_tensor("woT", [2, 128, C], bf16, kind="ExternalInput")
    bq_d = nc.dram_tensor("bq", [2, 128, 1], f32, kind="ExternalInput")
    bk_d = nc.dram_tensor("bk", [2, 128, 1], f32, kind="ExternalInput")
    bv_d = nc.dram_tensor("bv", [128, C], f32, kind="ExternalInput")
    bo_d = nc.dram_tensor("bo", [2, 128, 1], f32, kind="ExternalInput")
    gam_d = nc.dram_tensor("gamma", [2, 128, 1], f32, kind="ExternalInput")
    bet_d = nc.dram_tensor("beta", [2, 128, 1], f32, kind="ExternalInput")
    gsel_d = nc.dram_tensor("gsel", [128, GPC], f32, kind="ExternalInput")
    gselT_d = nc.dram_tensor("gselT", [GPC, 128], f32, kind="ExternalInput")
    out_d = nc.dram_tensor("out", [2, 128, QS], f32, kind="ExternalOutput")

    with tile.TileContext(nc) as tc, ExitStack() as ctx:
        const = ctx.enter_context(tc.tile_pool(name="const", bufs=1))
        big = ctx.enter_context(tc.tile_pool(name="big", bufs=1))
        sm = ctx.enter_context(tc.tile_pool(name="sm", bufs=2))
        mtp = ctx.enter_context(tc.tile_pool(name="mtp", bufs=3))
        pwp = ctx.enter_context(tc.tile_pool(name="pwp", bufs=4))
        bcp = ctx.enter_context(tc.tile_pool(name="bcp", bufs=2))
        ps_small = ctx.enter_context(
            tc.tile_pool(name="ps_small", bufs=2, space=bass.MemorySpace.PSUM))
        ps_s = ctx.enter_context(
            tc.tile_pool(name="ps_s", bufs=1, space=bass.MemorySpace.PSUM))
        ps_o = ctx.enter_context(
            tc.tile_pool(name="ps_o", bufs=1, space=bass.MemorySpace.PSUM))

        for _rep in range(reps):
            # ---- constants to SBUF ----
            def cload(dram, shape, dtype, tag):
                t = const.tile(shape, dtype, tag=tag, name=tag)
                nc.sync.dma_start(out=t[:], in_=dram[:])
                return t

            def cload2(dram, shape, dtype, tag):
                ts = []
                for ci in range(2):
                    t = const.tile(shape, dtype, tag=f"{tag}{ci}", name=f"{tag}{ci}")
                    nc.sync.dma_start(out=t[:], in_=dram[ci])
                    ts.append(t)
                return ts

            wqT = cload2(wqT_d, [128, C], bf16, "wqT")
            wkT = cload2(wkT_d, [128, C], bf16, "wkT")
            wvT = cload2(wvT_d, [128, C], bf16, "wvT")
            woT = cload2(woT_d, [128, C], bf16, "woT")
            bq = cload2(bq_d, [128, 1], f32, "bq")
            bk = cload2(bk_d, [128, 1], f32, "bk")
            bo = cload2(bo_d, [128, 1], f32, "bo")
            gam = cload2(gam_d, [128, 1], f32, "gamma")
            bet = cload2(bet_d, [128, 1], f32, "beta")
            bv = cload(bv_d, [128, C], f32, "bv")
            gsel = cload(gsel_d, [128, GPC], f32, "gsel")
            gselT = cload(gselT_d, [GPC, 128], f32, "gselT")
            ones = const.tile([1, 256], bf16, tag="ones")
            nc.vector.memset(ones[:], 1.0)
            epsb = const.tile([GPC, 1], f32, tag="epsb")
            nc.vector.memset(epsb[:], EPS)
            ones_f = const.tile([1, 64], f32, tag="ones_f")
            nc.vector.memset(ones_f[:], 1.0)

            def big2(shape, dtype, tag):
                return [big.tile(shape, dtype, tag=f"{tag}{ci}", name=f"{tag}{ci}")
                        for ci in range(2)]

            x_sb = big2([128, NQ], f32, "x")
            xq_sb = big2([128, QS], f32, "xq")
            for ci in range(2):
                nc.sync.dma_start(out=x_sb[ci][:], in_=x_d[ci])
                nc.sync.dma_start(out=xq_sb[ci][:], in_=xq_d[ci])

            h_sb = big2([128, NQ], bf16, "h")
            hq_sb = big2([128, QS], bf16, "hq")
            k_sb = big2([128, NQ], bf16, "k")
            q_sb = big2([128, QS], bf16, "q")
            vt_sb = big.tile([128, NJC * 260], bf16, tag="vt")
            at_sb = big2([128, QS], bf16, "at")
            out_sb = big2([128, QS], f32, "outs")
            scratch = big.tile([128, NQ], bf16, tag="scr")

            LVL = {"s0": 0, "s1": 1, "s2": 2, "s3": 3, "front": 4, "h1": 5,
                   "noaccum": 6, "full": 6}[stage]

            if LVL == 0:
                for co in range(2):
                    nc.vector.tensor_copy(out_sb[co][:], xq_sb[co][:])
                for co in range(2):
                    nc.sync.dma_start(out=out_d[co], in_=out_sb[co][:])
                return

            # ================= GroupNorm =================
            for ci in range(2):
                stats = sm.tile([128, 2], f32, tag="stats")
                nc.vector.tensor_reduce(
                    stats[:, 0:1], x_sb[ci][:], axis=mybir.AxisListType.X, op=OP.add)
                if USE_TTR:
                    nc.vector.tensor_tensor_reduce(
                        out=scratch[:, 0:NQ], in0=x_sb[ci][:], in1=x_sb[ci][:],
                        scale=1.0, scalar=0.0, op0=OP.mult, op1=OP.add,
                        accum_out=stats[:, 1:2])
                else:
                    nc.vector.tensor_tensor(scratch[:, 0:NQ], x_sb[ci][:],
                                            x_sb[ci][:], OP.mult)
                    nc.vector.tensor_reduce(
                        stats[:, 1:2], scratch[:, 0:NQ],
                        axis=mybir.AxisListType.X, op=OP.add)
                gstat = ps_small.tile([GPC, 2], f32, tag="pp")
                nc.tensor.matmul(gstat[:], gsel[:], stats[:], start=True, stop=True)
                gm = sm.tile([GPC, 2], f32, tag="gm")
                n_inv = 1.0 / (CPG * NQ)
                nc.vector.tensor_scalar(
                    gm[:], gstat[:], n_inv, None, op0=OP.mult)
                var = sm.tile([GPC, 2], f32, tag="var")
                # var[:,0] = E[x^2] - mean^2 ; var[:,1] = mean (carried along)
                nc.vector.tensor_tensor(var[:, 1:2], gm[:, 0:1], gm[:, 0:1], OP.mult)
                nc.vector.tensor_tensor(var[:, 0:1], gm[:, 1:2], var[:, 1:2],
                                        OP.subtract)
                std = sm.tile([GPC, 2], f32, tag="std")
                if USE_SQRT:
                    nc.scalar.activation(std[:, 0:1], var[:, 0:1], FT.Sqrt,
                                         bias=epsb[:])
                else:
                    nc.vector.tensor_copy(std[:, 0:1], var[:, 0:1])
                rs2 = sm.tile([GPC, 2], f32, tag="rs2")
                if USE_RECIP:
                    nc.vector.reciprocal(rs2[:, 1:2], std[:, 0:1])
                else:
                    nc.vector.tensor_copy(rs2[:, 1:2], std[:, 0:1])
                nc.vector.tensor_copy(rs2[:, 0:1], gm[:, 0:1])
                # broadcast to channels: chst[c, 0]=mean_c, chst[c, 1]=rstd_c
                chst = ps_small.tile([128, 2], f32, tag="pp")
                nc.tensor.matmul(chst[:], gselT[:], rs2[:], start=True, stop=True)
                scl = sm.tile([128, 1], f32, tag="scl")
                nc.vector.tensor_tensor(scl[:], chst[:, 1:2], gam[ci][:], OP.mult)
                mscl = sm.tile([128, 1], f32, tag="mscl")
                nc.vector.tensor_tensor(mscl[:], chst[:, 0:1], scl[:], OP.mult)
                bias_c = sm.tile([128, 1], f32, tag="biasc")
                nc.vector.tensor_tensor(bias_c[:], bet[ci][:], mscl[:], OP.subtract)
                nc.vector.tensor_scalar(
                    h_sb[ci][:], x_sb[ci][:], scl[:], bias_c[:], op0=OP.mult, op1=OP.add)
                nc.vector.tensor_scalar(
                    hq_sb[ci][:], xq_sb[ci][:], scl[:], bias_c[:], op0=OP.mult, op1=OP.add)

            if LVL == 1:
                for co in range(2):
                    nc.vector.tensor_copy(out_sb[co][:], hq_sb[co][:])
                for co in range(2):
                    nc.sync.dma_start(out=out_d[co], in_=out_sb[co][:])
                return

            # ================= Projections =================
            # q/k: [c_out, s] = sum_ci wT[ci][:, c_out].T @ h[ci][:, s]
            for co in range(2):
                for s in range(NQP):
                    pq = ps_small.tile([128, 512], f32, tag="pp")
                    for ci in range(2):
                        nc.tensor.matmul(
                            pq[:], wqT[ci][:, co * 128:(co + 1) * 128],
                            hq_sb[ci][:, s * 512:(s + 1) * 512],
                            start=(ci == 0), stop=(ci == 1))
                    nc.vector.tensor_scalar(
                        q_sb[co][:, s * 512:(s + 1) * 512], pq[:], bq[co][:], None,
                        op0=OP.add)
                for s in range(NQ // 512):
                    pk = ps_small.tile([128, 512], f32, tag="pp")
                    for ci in range(2):
                        nc.tensor.matmul(
                            pk[:], wkT[ci][:, co * 128:(co + 1) * 128],
                            h_sb[ci][:, s * 512:(s + 1) * 512],
                            start=(ci == 0), stop=(ci == 1))
                    nc.vector.tensor_scalar(
                        k_sb[co][:, s * 512:(s + 1) * 512], pk[:], bk[co][:], None,
                        op0=OP.add)

            if LVL == 2:
                for co in range(2):
                    nc.vector.tensor_copy(out_sb[co][:], q_sb[co][:])
                for co in range(2):
                    nc.sync.dma_start(out=out_d[co], in_=out_sb[co][:])
                return

            # vT: [j, c_out] = sum_ci h[ci][:, j].T @ wvT[ci]  (+ ones.T @ bv)
            nc.vector.memset(vt_sb[:], 1.0)
            for jc in range(NJC):
                pv = ps_small.tile([128, C], f32, tag="pp")
                for ci in range(2):
                    nc.tensor.matmul(
                        pv[:], h_sb[ci][:, jc * 128:(jc + 1) * 128], wvT[ci][:],
                        start=(ci == 0), stop=(ci == 1))
                base = jc * 260
                vt_view = vt_sb[:, base:base + 260].rearrange(
                    "p (h c) -> p h c", h=HEADS)[:, :, 0:64]
                pv_view = pv[:].rearrange("p (h c) -> p h c", h=HEADS)
                bv_view = bv[:].rearrange("p (h c) -> p h c", h=HEADS)
                nc.vector.tensor_tensor(vt_view, pv_view, bv_view, OP.add)

            if LVL == 3:
                for co in range(2):
                    nc.vector.tensor_copy(out_sb[co][:], vt_sb[:, co * 1024:(co + 1) * 1024])
                for co in range(2):
                    nc.sync.dma_start(out=out_d[co], in_=out_sb[co][:])
                return

            if stage in ("front", "h1"):
                for ci in range(2):
                    nc.vector.memset(at_sb[ci][:], 0.0)

            # ============ Attention (1024-wide, head pairs) ============
            nheads = 0 if stage == "front" else (1 if stage == "h1" else HEADS)
            accum = stage not in ("noaccum",)
            for hp in range((nheads + 1) // 2):
                hpair = [h for h in (hp * 2, hp * 2 + 1) if h < nheads]
                po = [ps_o.tile([65, QS], f32, tag=f"o{i}", name=f"o{i}")
                      for i in range(len(hpair))]
                for jc in range(NJC):
                    mt_t = mtp.tile([128, QS], bf16, tag="mt")
                    nc.sync.dma_start(
                        out=mt_t[:],
                        in_=mt_d[jc * 128:(jc + 1) * 128, :])
                    for i, h in enumerate(hpair):
                        ps = ps_s.tile([128, QS], f32, tag="s")
                        for qp in range(NQP):
                            nc.tensor.matmul(
                                ps[:, qp * 512:(qp + 1) * 512],
                                k_sb[h // 2][(h % 2) * 64:(h % 2) * 64 + 64,
                                     jc * 128:(jc + 1) * 128],
                                q_sb[h // 2][(h % 2) * 64:(h % 2) * 64 + 64,
                                     qp * 512:(qp + 1) * 512],
                                start=True, stop=True)
                        pt = pwp.tile([128, QS], bf16, tag="p")
                        nc.scalar.activation(pt[:], ps[:], FT.Exp)
                        wt = pwp.tile([128, QS], bf16, tag="w")
                        nc.vector.tensor_tensor(wt[:], pt[:], mt_t[:], OP.mult)
                        base = jc * 260 + h * 65
                        for qp in range(NQP):
                            nc.tensor.matmul(
                                po[i][:, qp * 512:(qp + 1) * 512],
                                vt_sb[:, base:base + 65],
                                wt[:, qp * 512:(qp + 1) * 512],
                                start=(jc == 0) if accum else True,
                                stop=(jc == NJC - 1) if accum else True)
                for i, h in enumerate(hpair):
                    rz = bcp.tile([1, QS], f32, tag="rz")
                    nc.vector.reciprocal(rz[:], po[i][64:65, :])
                    for qp in range(NQP):
                        pbc = ps_small.tile([128, 512], f32, tag="pp")
                        nc.tensor.matmul(
                            pbc[0:64, :], ones_f[:, 0:64],
                            rz[:, qp * 512:(qp + 1) * 512],
                            start=True, stop=True)
                        bc = bcp.tile([64, 512], f32, tag="bc")
                        nc.vector.tensor_copy(bc[:], pbc[0:64, :])
                        nc.vector.tensor_tensor(
                            at_sb[h // 2][(h % 2) * 64:(h % 2) * 64 + 64,
                                  qp * 512:(qp + 1) * 512],
                            po[i][0:64, qp * 512:(qp + 1) * 512], bc[:],
                            OP.mult)

            # ================= Output projection + residual =================
            for co in range(2):
                for qp in range(NQP):
                    pout = ps_small.tile([128, 512], f32, tag="pp")
                    for ci in range(2):
                        nc.tensor.matmul(
                            pout[:], woT[ci][:, co * 128:(co + 1) * 128],
                            at_sb[ci][:, qp * 512:(qp + 1) * 512],
                            start=(ci == 0), stop=(ci == 1))
                    nc.vector.scalar_tensor_tensor(
                        out_sb[co][:, qp * 512:(qp + 1) * 512], pout[:], bo[co][:],
                        xq_sb[co][:, qp * 512:(qp + 1) * 512],
                        op0=OP.add, op1=OP.add)
            for co in range(2):
                nc.sync.dma_start(out=out_d[co], in_=out_sb[co][:])


def _prep_in_maps(x, valid_indices_mask, attendable_indices, gn_scale, gn_bias,
                  wq, bq, wk, bk, wv, bv, wo, bo):
    x = np.ascontiguousarray(np.asarray(x, np.float32).reshape(B, C, NQ))
    idx = np.asarray(attendable_indices, np.int64)
    msk = np.asarray(valid_indices_mask, np.float64)

    # MT[j, q] = multiplicity of key j among valid slots of query q
    qcol = np.arange(NQ, dtype=np.int64)[:, None]
    flat = (idx * NQ + qcol).ravel()
    cnt = np.bincount(flat, weights=msk.ravel(), minlength=NQ * NQ)
    MT = cnt.reshape(NQ, NQ).astype(ml_dtypes.bfloat16)

    def chunk_w(w):
        return np.ascontiguousarray(
            np.asarray(w, np.float32).T.reshape(2, 128, C)
        ).astype(ml_dtypes.bfloat16)

    def chunk_b(b):
        return np.ascontiguousarray(
            np.asarray(b, np.float32).reshape(2, 128, 1))

    gsel = np.zeros((128, GPC), np.float32)
    gsel[np.arange(128), np.arange(128) // CPG] = 1.0

    # reference reshapes attn [b, heads, nq, d] -> transpose -> [b, d, heads, nq]
    # -> channels c = d*HEADS + head.  Our attn rows are head-major
    # (r = head*64 + d), so permute wo's input columns to compensate.
    r = np.arange(C)
    perm = (r % D) * HEADS + (r // D)
    wo_p = np.asarray(wo, np.float32)[:, perm]
    shared = {
        "wqT": chunk_w(wq), "wkT": chunk_w(wk), "wvT": chunk_w(wv),
        "woT": chunk_w(wo_p),
        "bq": chunk_b(bq), "bk": chunk_b(bk), "bo": chunk_b(bo),
        "bv": np.broadcast_to(
            np.asarray(bv, np.float32).reshape(1, C), (128, C)).copy(),
        "gamma": chunk_b(gn_scale), "beta": chunk_b(gn_bias),
        "gsel": gsel, "gselT": np.ascontiguousarray(gsel.T),
    }
    in_maps = []
    for core in range(NCORES):
        b = core // (NCORES // B)
        qoff = (core % (NCORES // B)) * QS
        xb = x[b].reshape(2, 128, NQ)
        m = dict(shared)
        m["x"] = np.ascontiguousarray(xb)
        m["xq"] = np.ascontiguousarray(xb[:, :, qoff:qoff + QS])
        m["mt"] = np.ascontiguousarray(MT[:, qoff:qoff + QS])
        in_maps.append(m)
    return in_maps


def _execute(in_maps, trace=False, stage="full", reps=1):
    key = f"nc_{stage}_{reps}"
    if key not in _CACHE:
        _CACHE[key] = _build_program(stage, reps)
    return run_bass_kernel_spmd(
        _CACHE[key], in_maps, list(range(NCORES)), trace=trace)


def _assemble(results):
    out = np.zeros((B, C, NQ), np.float32)
    for core in range(NCORES):
        b = core // (NCORES // B)
        qoff = (core % (NCORES // B)) * QS
        o = results[core]["out"]  # [2, 128, QS]
        out[b, :, qoff:qoff + QS] = o.reshape(C, QS)
    return out.reshape(B, C, HI, WI)


def kernel(**inputs):
    in_maps = _prep_in_maps(**inputs)
    res = _execute(in_maps, trace=False)
    return _assemble(res.results)

